# revision 4
# baseline (speedup 1.0000x reference)
"""Trainium2 Bass kernel for nn_ContextLayer (gnn_message_passing).

Math (reference):
  g0 = x @ W0.T + b0            [B,S,D]
  g1 = x @ W1.T + b1            [B,S,D]
  out[b,q,e] = tanh( (1/L_b) * sum_k m[b,q] m[b,k] x[b,k,e] sigmoid(g0[b,q,e]+g1[b,k,e]) )

Sharding: 8 cores = 4 batches x 2 e-halves (200 e's each). Each core:
  - computes g0t/g1t = [e, s] gate matrices via PE matmuls (contraction over
    d as partitions, 4 K-chunks of 401 rows: 400 features + 1 mask-penalty
    row that adds BIGNEG*(1-m[s]) so masked s give sigmoid()==0),
  - inner loop over (e-chunk, q): ACT computes sigmoid(g1t + g0t[:,q]) with
    the per-partition bias port (fused add), DVE tensor_tensor_reduce fuses
    the multiply by x[k,e] and the k-reduction into acc[:, q],
  - final tanh(acc * 1/L) on ACT with the per-partition scale port.

Host side only slices/transposes inputs and assembles the output.
"""

import numpy as np
from contextlib import ExitStack

from concourse import bacc, mybir, tile
import concourse.bass as bass
from concourse.bass_utils import run_bass_kernel_spmd

B, S, D = 4, 256, 400
EH = 200                      # e-columns per core
CHUNKS = [(0, 128), (128, 72)]  # (e-local offset, partitions)
KCH = [(0, 128), (128, 128), (256, 128), (384, 17)]  # K-chunks over 401
BIGNEG = np.float32(-1e30)
F32 = mybir.dt.float32
N_CORES = 8

_prog_cache = {}


def _build_program(repeat=1):
    nc = bacc.Bacc("TRN2", target_bir_lowering=False, debug=False)

    xin = nc.dram_tensor("xin", [401, 256], F32, kind="ExternalInput").ap()
    xtin = nc.dram_tensor("xtin", [200, 256], F32, kind="ExternalInput").ap()
    w0t = nc.dram_tensor("w0t", [401, 200], F32, kind="ExternalInput").ap()
    w1t = nc.dram_tensor("w1t", [401, 200], F32, kind="ExternalInput").ap()
    bias01 = nc.dram_tensor("bias01", [128, 4], F32, kind="ExternalInput").ap()
    invl = nc.dram_tensor("invl", [128, 1], F32, kind="ExternalInput").ap()
    out = nc.dram_tensor("out", [200, 256], F32, kind="ExternalOutput").ap()

    AF = mybir.ActivationFunctionType
    OP = mybir.AluOpType

    with ExitStack() as ctx:
        tc = ctx.enter_context(tile.TileContext(nc))
        if repeat > 1:
            ctx.enter_context(tc.For_i(0, repeat, 1))
        const = ctx.enter_context(tc.tile_pool(name="const", bufs=1))
        psum = ctx.enter_context(tc.tile_pool(name="psum", bufs=1, space="PSUM"))
        tpool = ctx.enter_context(tc.tile_pool(name="t", bufs=8))

        # ---- loads ----
        rhs = []
        for k0, kn in KCH:
            t = const.tile([kn, 256], F32, tag=f"rhs{k0}")
            nc.sync.dma_start(t[:], xin[k0 : k0 + kn, :])
            rhs.append(t)
        wts = []
        for gi, wsrc in enumerate([w0t, w1t]):
            chunks = []
            for k0, kn in KCH:
                t = const.tile([kn, 200], F32, tag=f"w{gi}_{k0}")
                nc.sync.dma_start(t[:], wsrc[k0 : k0 + kn, :])
                chunks.append(t)
            wts.append(chunks)
        biases = const.tile([128, 4], F32, tag="biases")
        nc.sync.dma_start(biases[:], bias01[:])
        invlt = const.tile([128, 1], F32, tag="invlt")
        nc.sync.dma_start(invlt[:], invl[:])
        xt = []
        for ci, (e0, pn) in enumerate(CHUNKS):
            t = const.tile([pn, 256], F32, tag=f"xt{ci}")
            nc.sync.dma_start(t[:], xtin[e0 : e0 + pn, :])
            xt.append(t)

        # ---- gates: g{0,1}t[e_chunk, s] = W.T @ x.T (+bias, +mask penalty) ----
        gt = [[None, None], [None, None]]  # [gi][ci]
        for ci, (e0, pn) in enumerate(CHUNKS):
            for gi in range(2):
                ps = psum.tile([pn, 256], F32, tag=f"ps{gi}{ci}")
                for kci, (k0, kn) in enumerate(KCH):
                    nc.tensor.matmul(
                        ps[:],
                        wts[gi][kci][:, e0 : e0 + pn],
                        rhs[kci][:],
                        start=(kci == 0),
                        stop=(kci == len(KCH) - 1),
                    )
                gs = const.tile([pn, 256], F32, tag=f"g{gi}t{ci}")
                nc.scalar.activation(
                    gs[:], ps[:], AF.Identity,
                    bias=biases[0:pn, 2 * gi + ci : 2 * gi + ci + 1],
                )
                gt[gi][ci] = gs

        # ---- main loop: acc[e, q] = sum_k sigmoid(g1t[e,k] + g0t[e,q]) * x[k,e] ----
        accs = []
        for ci, (e0, pn) in enumerate(CHUNKS):
            acc = const.tile([pn, 256], F32, tag=f"acc{ci}")
            scratch = const.tile([pn, 256], F32, tag=f"scr{ci}")
            for q in range(256):
                tt = tpool.tile([pn, 256], F32, tag=f"t{ci}")
                nc.scalar.activation(
                    tt[:], gt[1][ci][:], AF.Sigmoid,
                    bias=gt[0][ci][:, q : q + 1],
                )
                nc.vector.tensor_tensor(
                    out=scratch[:], in0=tt[:], in1=xt[ci][:], op=OP.mult
                )
                nc.vector.tensor_reduce(
                    out=acc[:, q : q + 1],
                    in_=scratch[:],
                    axis=mybir.AxisListType.X,
                    op=OP.add,
                )
            accs.append(acc)

        # ---- finalize: out = tanh(acc / L) ----
        for ci, (e0, pn) in enumerate(CHUNKS):
            res = const.tile([pn, 256], F32, tag=f"res{ci}")
            nc.scalar.activation(
                res[:], accs[ci][:], AF.Tanh, scale=invlt[0:pn, :]
            )
            nc.sync.dma_start(out[e0 : e0 + pn, :], res[:])

    nc.compile()
    return nc


def _get_program():
    if "nc" not in _prog_cache:
        _prog_cache["nc"] = _build_program()
    return _prog_cache["nc"]


def _make_in_maps(x, m, W0, b0, W1, b1):
    maskrow = (1.0 - m).astype(np.float32)  # [B, S]
    L = m.sum(axis=1)
    invL = np.where(L > 0, 1.0 / np.maximum(L, 1.0), np.float32(np.inf)).astype(
        np.float32
    )
    w_aug = []
    for W in (W0, W1):
        w_aug.append(
            np.concatenate(
                [np.ascontiguousarray(W.T), np.full((1, D), BIGNEG, np.float32)], 0
            )
        )
    in_maps = []
    for c in range(N_CORES):
        b, h = c // 2, c % 2
        e0 = EH * h
        xT = np.ascontiguousarray(x[b].T)  # [400, 256]
        xin = np.concatenate([xT, maskrow[b][None, :]], 0)  # [401, 256]
        bias01 = np.zeros((128, 4), np.float32)
        bias01[:128, 0] = b0[e0 : e0 + 128]
        bias01[:72, 1] = b0[e0 + 128 : e0 + 200]
        bias01[:128, 2] = b1[e0 : e0 + 128]
        bias01[:72, 3] = b1[e0 + 128 : e0 + 200]
        in_maps.append(
            {
                "xin": np.ascontiguousarray(xin),
                "xtin": np.ascontiguousarray(xT[e0 : e0 + EH]),
                "w0t": np.ascontiguousarray(w_aug[0][:, e0 : e0 + EH]),
                "w1t": np.ascontiguousarray(w_aug[1][:, e0 : e0 + EH]),
                "bias01": bias01,
                "invl": np.full((128, 1), invL[b], np.float32),
            }
        )
    return in_maps


def run(inputs, trace=False, trace_kwargs=None):
    """Run on hardware; returns (output, BassKernelResults)."""
    x = np.asarray(inputs["input"], np.float32)
    m = np.asarray(inputs["input_masks"]).astype(np.float32)
    W0 = np.asarray(inputs["W0"], np.float32)
    b0 = np.asarray(inputs["b0"], np.float32)
    W1 = np.asarray(inputs["W1"], np.float32)
    b1 = np.asarray(inputs["b1"], np.float32)

    in_maps = _make_in_maps(x, m, W0, b0, W1, b1)
    nc = _get_program()
    kw = dict(trace=trace)
    if trace_kwargs:
        kw.update(trace_kwargs)
    res = run_bass_kernel_spmd(nc, in_maps, list(range(N_CORES)), **kw)

    out = np.empty((B, S, D), np.float32)
    for c in range(N_CORES):
        b, h = c // 2, c % 2
        out[b, :, EH * h : EH * h + EH] = res.results[c]["out"].T
    return out, res


def kernel(input, input_masks, W0, b0, W1, b1):
    out, _ = run(
        {
            "input": input,
            "input_masks": input_masks,
            "W0": W0,
            "b0": b0,
            "W1": W1,
            "b1": b1,
        }
    )
    return out


# revision 6
# speedup vs baseline: 6.8498x; 6.8498x over previous
"""Trainium2 Bass kernel for nn_ContextLayer (gnn_message_passing).

Math (reference):
  g0 = x @ W0.T + b0            [B,S,D]
  g1 = x @ W1.T + b1            [B,S,D]
  out[b,q,e] = tanh( (1/L_b) * sum_k m[b,q] m[b,k] x[b,k,e] sigmoid(g0[b,q,e]+g1[b,k,e]) )

Sharding: 8 cores = 4 batches x 2 e-halves (200 e's each). Each core:
  - computes g0t/g1t = [e, s] gate matrices via PE matmuls (contraction over
    d as partitions, 4 K-chunks of 401 rows: 400 features + 1 mask-penalty
    row that adds BIGNEG*(1-m[s]) so masked s give sigmoid()==0),
  - inner loop over (e-chunk, q): ACT computes sigmoid(g1t + g0t[:,q]) with
    the per-partition bias port (fused add), DVE tensor_tensor_reduce fuses
    the multiply by x[k,e] and the k-reduction into acc[:, q],
  - final tanh(acc * 1/L) on ACT with the per-partition scale port.

Host side only slices/transposes inputs and assembles the output.
"""

import numpy as np
from contextlib import ExitStack

from concourse import bacc, mybir, tile
import concourse.bass as bass
from concourse.bass_utils import run_bass_kernel_spmd

B, S, D = 4, 256, 400
EH = 200                      # e-columns per core
CHUNKS = [(0, 128), (128, 72)]  # (e-local offset, partitions)
KCH = [(0, 128), (128, 128), (256, 128), (384, 17)]  # K-chunks over 401
BIGNEG = np.float32(-1e30)
F32 = mybir.dt.float32
N_CORES = 8

_prog_cache = {}


def _build_program(repeat=1):
    nc = bacc.Bacc("TRN2", target_bir_lowering=False, debug=False)

    xin = nc.dram_tensor("xin", [401, 256], F32, kind="ExternalInput").ap()
    xtin = nc.dram_tensor("xtin", [200, 256], F32, kind="ExternalInput").ap()
    w0t = nc.dram_tensor("w0t", [401, 200], F32, kind="ExternalInput").ap()
    w1t = nc.dram_tensor("w1t", [401, 200], F32, kind="ExternalInput").ap()
    bias01 = nc.dram_tensor("bias01", [128, 4], F32, kind="ExternalInput").ap()
    invl = nc.dram_tensor("invl", [128, 1], F32, kind="ExternalInput").ap()
    out = nc.dram_tensor("out", [200, 256], F32, kind="ExternalOutput").ap()

    AF = mybir.ActivationFunctionType
    OP = mybir.AluOpType

    with ExitStack() as ctx:
        tc = ctx.enter_context(tile.TileContext(nc))
        if repeat > 1:
            ctx.enter_context(tc.For_i(0, repeat, 1))
        const = ctx.enter_context(tc.tile_pool(name="const", bufs=1))
        psum = ctx.enter_context(tc.tile_pool(name="psum", bufs=1, space="PSUM"))
        tpool = ctx.enter_context(tc.tile_pool(name="t", bufs=3))

        # ---- loads ----
        rhs = []
        for k0, kn in KCH:
            t = const.tile([kn, 256], F32, tag=f"rhs{k0}")
            nc.sync.dma_start(t[:], xin[k0 : k0 + kn, :])
            rhs.append(t)
        wts = []
        for gi, wsrc in enumerate([w0t, w1t]):
            chunks = []
            for k0, kn in KCH:
                t = const.tile([kn, 200], F32, tag=f"w{gi}_{k0}")
                nc.sync.dma_start(t[:], wsrc[k0 : k0 + kn, :])
                chunks.append(t)
            wts.append(chunks)
        biases = const.tile([128, 4], F32, tag="biases")
        nc.sync.dma_start(biases[:], bias01[:])
        invlt = const.tile([128, 1], F32, tag="invlt")
        nc.sync.dma_start(invlt[:], invl[:])
        xt = []
        for ci, (e0, pn) in enumerate(CHUNKS):
            t = const.tile([pn, 256], F32, tag=f"xt{ci}")
            nc.sync.dma_start(t[:], xtin[e0 : e0 + pn, :])
            xt.append(t)

        # ---- gates: g{0,1}t[e_chunk, s] = W.T @ x.T (+bias, +mask penalty) ----
        gt = [[None, None], [None, None]]  # [gi][ci]
        for ci, (e0, pn) in enumerate(CHUNKS):
            for gi in range(2):
                ps = psum.tile([pn, 256], F32, tag=f"ps{gi}{ci}")
                for kci, (k0, kn) in enumerate(KCH):
                    nc.tensor.matmul(
                        ps[:],
                        wts[gi][kci][:, e0 : e0 + pn],
                        rhs[kci][:],
                        start=(kci == 0),
                        stop=(kci == len(KCH) - 1),
                    )
                gs = const.tile([pn, 256], F32, tag=f"g{gi}t{ci}")
                nc.scalar.activation(
                    gs[:], ps[:], AF.Identity,
                    bias=biases[0:pn, 2 * gi + ci : 2 * gi + ci + 1],
                )
                gt[gi][ci] = gs

        # ---- main loop: acc[e, q] = sum_k sigmoid(g1t[e,k] + g0t[e,q]) * x[k,e] ----
        # Per q-block of QB: QB biased sigmoids (ACT) into a wide tile, one
        # big multiply vs broadcast x (DVE or Pool), one segmented reduce (DVE).
        QB = 8
        NBLK = 256 // QB
        # DVE takes DVE_BLKS of every 16 blocks' multiplies, Pool the rest
        DVE_MUL = {0, 3, 6, 9, 12}
        accs = []
        for ci, (e0, pn) in enumerate(CHUNKS):
            acc = const.tile([pn, 256], F32, tag=f"acc{ci}")
            xt_b = (
                xt[ci][:]
                .rearrange("p (o k) -> p o k", o=1)
                .broadcast_to((pn, QB, 256))
            )
            for bi in range(NBLK):
                tw = tpool.tile([pn, QB * 256], F32, tag=f"tw{ci}")
                for j in range(QB):
                    q = bi * QB + j
                    nc.scalar.activation(
                        tw[:, j * 256 : (j + 1) * 256], gt[1][ci][:],
                        AF.Sigmoid, bias=gt[0][ci][:, q : q + 1],
                    )
                prod = tpool.tile([pn, QB * 256], F32, tag=f"prod{ci}")
                tw3 = tw[:].rearrange("p (q k) -> p q k", q=QB)
                prod3 = prod[:].rearrange("p (q k) -> p q k", q=QB)
                if (bi % 16) in DVE_MUL:
                    nc.vector.tensor_tensor(out=prod3, in0=tw3, in1=xt_b, op=OP.mult)
                else:
                    nc.gpsimd.tensor_tensor(out=prod3, in0=tw3, in1=xt_b, op=OP.mult)
                nc.vector.tensor_reduce(
                    out=acc[:, bi * QB : (bi + 1) * QB],
                    in_=prod3,
                    axis=mybir.AxisListType.X,
                    op=OP.add,
                )
            accs.append(acc)

        # ---- finalize: out = tanh(acc / L) ----
        for ci, (e0, pn) in enumerate(CHUNKS):
            res = const.tile([pn, 256], F32, tag=f"res{ci}")
            nc.scalar.activation(
                res[:], accs[ci][:], AF.Tanh, scale=invlt[0:pn, :]
            )
            nc.sync.dma_start(out[e0 : e0 + pn, :], res[:])

    nc.compile()
    return nc


def _get_program():
    if "nc" not in _prog_cache:
        _prog_cache["nc"] = _build_program()
    return _prog_cache["nc"]


def _make_in_maps(x, m, W0, b0, W1, b1):
    maskrow = (1.0 - m).astype(np.float32)  # [B, S]
    L = m.sum(axis=1)
    invL = np.where(L > 0, 1.0 / np.maximum(L, 1.0), np.float32(np.inf)).astype(
        np.float32
    )
    w_aug = []
    for W in (W0, W1):
        w_aug.append(
            np.concatenate(
                [np.ascontiguousarray(W.T), np.full((1, D), BIGNEG, np.float32)], 0
            )
        )
    in_maps = []
    for c in range(N_CORES):
        b, h = c // 2, c % 2
        e0 = EH * h
        xT = np.ascontiguousarray(x[b].T)  # [400, 256]
        xin = np.concatenate([xT, maskrow[b][None, :]], 0)  # [401, 256]
        bias01 = np.zeros((128, 4), np.float32)
        bias01[:128, 0] = b0[e0 : e0 + 128]
        bias01[:72, 1] = b0[e0 + 128 : e0 + 200]
        bias01[:128, 2] = b1[e0 : e0 + 128]
        bias01[:72, 3] = b1[e0 + 128 : e0 + 200]
        in_maps.append(
            {
                "xin": np.ascontiguousarray(xin),
                "xtin": np.ascontiguousarray(xT[e0 : e0 + EH]),
                "w0t": np.ascontiguousarray(w_aug[0][:, e0 : e0 + EH]),
                "w1t": np.ascontiguousarray(w_aug[1][:, e0 : e0 + EH]),
                "bias01": bias01,
                "invl": np.full((128, 1), invL[b], np.float32),
            }
        )
    return in_maps


def run(inputs, trace=False, trace_kwargs=None):
    """Run on hardware; returns (output, BassKernelResults)."""
    x = np.asarray(inputs["input"], np.float32)
    m = np.asarray(inputs["input_masks"]).astype(np.float32)
    W0 = np.asarray(inputs["W0"], np.float32)
    b0 = np.asarray(inputs["b0"], np.float32)
    W1 = np.asarray(inputs["W1"], np.float32)
    b1 = np.asarray(inputs["b1"], np.float32)

    in_maps = _make_in_maps(x, m, W0, b0, W1, b1)
    nc = _get_program()
    kw = dict(trace=trace)
    if trace_kwargs:
        kw.update(trace_kwargs)
    res = run_bass_kernel_spmd(nc, in_maps, list(range(N_CORES)), **kw)

    out = np.empty((B, S, D), np.float32)
    for c in range(N_CORES):
        b, h = c // 2, c % 2
        out[b, :, EH * h : EH * h + EH] = res.results[c]["out"].T
    return out, res


def kernel(input, input_masks, W0, b0, W1, b1):
    out, _ = run(
        {
            "input": input,
            "input_masks": input_masks,
            "W0": W0,
            "b0": b0,
            "W1": W1,
            "b1": b1,
        }
    )
    return out
